# revision 41
# baseline (speedup 1.0000x reference)
"""
Trainium2 Bass kernel for nn_MultiHeadAttention_74586402062628.

Data-parallel across 8 NeuronCores: one batch element per core.

Per-core formulation (B=8, S=1000, E=1024, H=16, D=64), bf16 matmuls:
  - x [S,E] bf16 is transposed on-chip (PE transpose) to xT [E,S] bf16;
    the V projection runs per t-block right behind each block's
    transposes so the PE is dense from the start (HAM warm-up).
  - Q,K projections produce qT,kT [H*D, S] bf16 (head-dim on partitions;
    head h lives in 128-row tile h//2 at partition base (h%2)*64).
    Attention for the first s-tile of head pair m is emitted right after
    projection m so its exp() hides under the projection stretch.
  - V is scattered per-head into [v(64) | ones] slabs so the AV matmul's
    psum row 64 yields the softmax denominator for free.
  - Attention is computed transposed: scoresT[t,s] = k_t . q_s / sqrt(S).
    bf16 matmuls run at 1 cyc/row at ANY free dim, so every scores/AV
    tile is trimmed to exactly the unmasked columns (off = t0-s0). The
    causal diagonal is applied with a gpsimd affine_select (keep c >= p)
    directly on the exp tile; exp() without max-subtraction (logits are
    tiny for this problem).
  - Normalize: zp psum is evicted early to SBUF (frees the accumulator;
    fp32r so the denom row can feed the fp32r ones-matmul broadcast),
    1/denom via reciprocal_approx_fast on the broadcast, multiplies on
    GpSimd; odd heads partition-shift into yT via SBUF->SBUF DMA.
  - Output projection for s<512 is interleaved with second-s-tile
    attention (which is exp-bound on ScalarE); bias row added via a K=1
    ones matmul; exact GELU on ScalarE.
  - bv is folded into an effective output bias bpe = bp + bv @ wp (valid
    because softmax rows sum to 1); bq/bk are added on PSUM->SBUF
    eviction (fp32).
"""

import math
import os
import sys

for _p in ("/opt/trn_rl_repo", "/opt/pypackages"):
    if _p not in sys.path:
        sys.path.insert(0, _p)

import numpy as np

B, S, E, H, D = 8, 1000, 1024, 16, 64
P = 128
NB = 8                      # 128-row blocks covering S (last is partial)
LAST = S - (NB - 1) * P     # 104
KT = 8                      # 128-row contraction tiles covering E
ST = ((0, 512), (512, 488))     # s tiles (start, width) covering S
FT = ((0, 512), (512, 512))     # f/n tiles covering E
SCALE = 1.0 / math.sqrt(S)
NCORES = 8
PIPE = 2

# BASSMHA_NO_GELU=1: replace final GELU with Identity (CoreSim lacks Gelu)
_NO_GELU = os.environ.get("BASSMHA_NO_GELU", "0") == "1"

_CACHE = {}


def _build_nc():
    from concourse import bass, bacc
    import concourse.mybir as mybir
    from concourse import tile
    from concourse.masks import make_identity

    dt = mybir.dt
    f32 = dt.float32
    f32r = dt.float32r
    bf16 = dt.bfloat16
    AF = mybir.ActivationFunctionType
    Alu = mybir.AluOpType

    nc = bacc.Bacc("TRN2", debug=False, target_bir_lowering=False,
                   num_devices=NCORES)

    x_d = nc.declare_dram_parameter("x", [S, E], bf16, isOutput=False)
    wq_d = nc.declare_dram_parameter("wq2", [E, E], bf16, isOutput=False)
    wk_d = nc.declare_dram_parameter("wk2", [E, E], bf16, isOutput=False)
    wv_d = nc.declare_dram_parameter("wv2", [E, E], bf16, isOutput=False)
    wp_d = nc.declare_dram_parameter("wp2", [E, E], bf16, isOutput=False)
    bq_d = nc.declare_dram_parameter("bqt", [P, KT], f32, isOutput=False)
    bk_d = nc.declare_dram_parameter("bkt", [P, KT], f32, isOutput=False)
    bp_d = nc.declare_dram_parameter("bpe", [1, E], bf16, isOutput=False)
    out_d = nc.declare_dram_parameter("out", [S, E], f32, isOutput=True)

    def g2(ap):
        return ap.rearrange("p (g c) -> p g c", g=2)

    with tile.TileContext(nc) as tc:
        with (
            tc.tile_pool(name="const", bufs=1) as constp,
            tc.tile_pool(name="persist", bufs=1) as persist,
        ):
            # identity first on the gpsimd queue — it gates every transpose
            # and DMA triggers occupy the queue for the whole transfer
            ident = constp.tile([P, P], bf16)
            make_identity(nc, ident[:])
            # ones on every partition, f32r for the K=1 denom broadcast
            ones_r = constp.tile([P, P], f32r)
            nc.gpsimd.memset(ones_r[:].bitcast(f32), 1.0)
            # single ones row (partition 0) for the K=1 output-bias matmul
            ones_b = constp.tile([1, P], bf16)
            nc.gpsimd.memset(ones_b[:], 1.0)

            # Persistent activations
            qT = persist.tile([P, KT, S], bf16)      # [hd, m, s]
            kT = persist.tile([P, KT, S], bf16)
            v_e = persist.tile([P, H // 2, NB, 65], bf16)
            v_o = persist.tile([P, H // 2, NB, 65], bf16)
            yT = persist.tile([P, KT, S], bf16)      # normalized z, stacked

            # weights resident in SBUF for the whole kernel, spread across
            # the three DMA-capable queues so wv/wq/wk stream concurrently
            # (wv gates the V projection, wq the Q loop, wk trails)
            wq_sb = persist.tile([P, KT, E], bf16)
            wk_sb = persist.tile([P, KT, E], bf16)
            wv_sb = persist.tile([P, KT, E], bf16)
            wp_sb = persist.tile([P, KT, E], bf16)
            for k in range(KT):
                nc.sync.dma_start(wv_sb[:, k, :], wv_d[k * P:(k + 1) * P, :])
            for k in range(KT):
                nc.scalar.dma_start(wq_sb[:, k, :], wq_d[k * P:(k + 1) * P, :])
            for k in range(KT):
                nc.scalar.dma_start(wk_sb[:, k, :], wk_d[k * P:(k + 1) * P, :])
            for k in range(KT):
                nc.sync.dma_start(wp_sb[:, k, :], wp_d[k * P:(k + 1) * P, :])
            bq_sb = constp.tile([P, KT], f32)
            nc.sync.dma_start(bq_sb[:], bq_d[:, :])
            bk_sb = constp.tile([P, KT], f32)
            nc.sync.dma_start(bk_sb[:], bk_d[:, :])
            bp_sb = constp.tile([1, E], bf16)
            nc.sync.dma_start(bp_sb[:], bp_d[:, :])

            with (
                tc.tile_pool(name="xT", bufs=1) as xtp,
                tc.tile_pool(name="qkpsum", bufs=2, space="PSUM") as qkpsum,
            ):
                xT = xtp.tile([P, KT, S], bf16)

                # ---- Phase 1: transpose x -> xT ----
                with (
                    tc.tile_pool(name="xload", bufs=NB) as xload,
                    tc.tile_pool(name="tpsum", bufs=3, space="PSUM") as tpsum,
                ):
                    # x loads first on the gpsimd queue so they stream in
                    # parallel with the weight DMAs on the sync queue
                    xts = []
                    for sb in range(NB):
                        rows = LAST if sb == NB - 1 else P
                        xt = xload.tile([P, E], bf16, tag="xt",
                                        name=f"xt_{sb}")
                        nc.gpsimd.dma_start(xt[0:rows, :],
                                            x_d[sb * P:sb * P + rows, :])
                        xts.append(xt)
                    # init v slabs: ones columns everywhere, then zero the
                    # tb7 padding rows (96:128; the scatter rewrites rows
                    # 0:104 afterwards)
                    nc.gpsimd.memset(v_e[:, :, :, 64:65], 1.0)
                    nc.gpsimd.memset(v_o[:, :, :, 64:65], 1.0)
                    nc.gpsimd.memset(v_e[96:P, :, NB - 1, :], 0.0)
                    nc.gpsimd.memset(v_o[96:P, :, NB - 1, :], 0.0)
                    for sb in range(NB):
                        rows = LAST if sb == NB - 1 else P
                        t0 = sb * P
                        xt = xts[sb]
                        # all 8 transposes of this block share one psum bank;
                        # a single 2x-rate DVE copy evicts them together
                        tp = tpsum.tile([P, KT, P], bf16, tag="tp")
                        for eb in range(KT):
                            nc.tensor.transpose(
                                tp[0:P, eb, 0:rows],
                                xt[0:rows, eb * P:(eb + 1) * P],
                                ident[0:rows, 0:rows],
                            )
                        nc.vector.tensor_copy(
                            xT[:, :, t0:t0 + rows], tp[:, :, 0:rows])
                # ---- attention pools (live through phases 2-4) ----
                with (
                    tc.tile_pool(name="expp", bufs=5) as expp,
                    tc.tile_pool(name="zsbp", bufs=2) as zsbp,
                    tc.tile_pool(name="rcpp", bufs=2) as rcpp,
                    tc.tile_pool(name="ztop", bufs=2) as ztop,
                    tc.tile_pool(name="spsum", bufs=2, space="PSUM") as spsum,
                    tc.tile_pool(name="zpsum", bufs=1, space="PSUM") as zpsum,
                ):
                    def attn(s0, W, hp):
                        n_tb = (s0 + W + P - 1) // P
                        zp = zpsum.tile([P, 1024], f32, tag="zp")
                        exs = {}
                        geom = {}
                        for tb in range(n_tb):
                            rows = LAST if tb == NB - 1 else P
                            t0 = tb * P
                            off = t0 - s0 if t0 >= s0 else 0
                            geom[tb] = (rows, t0 >= s0, off, W - off)
                        for i in range(n_tb + PIPE):
                            if i < n_tb:
                                tb = i
                                rows, has_diag, off, N = geom[tb]
                                t0 = tb * P
                                sp = spsum.tile([P, 1024], f32, tag="sp")
                                for par in range(2):
                                    base = par * 64
                                    nc.tensor.matmul(
                                        sp[0:rows, 512 * par:512 * par + N],
                                        kT[base:base + 64, hp, t0:t0 + rows],
                                        qT[base:base + 64, hp,
                                           s0 + off:s0 + W],
                                        start=True, stop=True,
                                    )
                                ex = expp.tile([P, 1024], bf16, tag="ex")
                                exv, spv = g2(ex[:, :]), g2(sp[:, :])
                                if rows < P:
                                    nc.vector.memset(exv[96:P, :, 0:N], 0.0)
                                nc.scalar.activation(
                                    exv[0:rows, :, 0:N], spv[0:rows, :, 0:N],
                                    AF.Exp, scale=SCALE)
                                if has_diag:
                                    dw = min(rows, N)
                                    nc.gpsimd.affine_select(
                                        out=exv[0:rows, :, 0:dw],
                                        in_=exv[0:rows, :, 0:dw],
                                        compare_op=Alu.is_ge, fill=0.0,
                                        base=0, channel_multiplier=-1,
                                        pattern=[[0, 2], [1, dw]],
                                    )
                                exs[tb] = ex
                            j = i - PIPE
                            if 0 <= j < n_tb:
                                rows, has_diag, off, N = geom[j]
                                ex = exs.pop(j)
                                for par, vs in ((0, v_e), (1, v_o)):
                                    nc.tensor.matmul(
                                        zp[0:65,
                                           512 * par + off:512 * par + W],
                                        vs[:, hp, j, 0:65],
                                        ex[0:P, 512 * par:512 * par + N],
                                        start=(j == 0), stop=(j == n_tb - 1),
                                        skip_group_check=True,
                                    )
                        # normalize: evict zp to SBUF per parity (frees the
                        # accumulator; f32r so the denom row feeds the f32r
                        # ones-matmul), broadcast denom to partitions 0:64
                        # (per-parity psum tiles from the qkpsum pool so the
                        # zp ring never waits on this chain), fast
                        # reciprocal on the broadcast, multiply on GpSimd.
                        zsb = zsbp.tile([65, 1024], f32r, tag="zsb")
                        rcp = rcpp.tile([64, 1024], f32, tag="rcp")
                        zto = ztop.tile([64, 512], bf16, tag="zto")
                        for par in range(2):
                            c0 = 512 * par
                            with nc.allow_low_precision(
                                    reason="z/denom round to fp32r for the "
                                    "broadcast matmul; within tolerance"):
                                nc.vector.tensor_copy(
                                    zsb[0:65, c0:c0 + W],
                                    zp[0:65, c0:c0 + W])
                            bc = qkpsum.tile([P, 512], f32, tag="ps",
                                             name=f"bc_{hp}_{s0}_{par}")
                            nc.tensor.matmul(
                                bc[0:64, 0:W],
                                ones_r[64:65, 0:64],
                                zsb[64:65, c0:c0 + W],
                                start=True, stop=True)
                            nc.vector.reciprocal_approx_fast(
                                rcp[0:64, c0:c0 + W], bc[0:64, 0:W])
                        nc.gpsimd.tensor_tensor(
                            yT[0:64, hp, s0:s0 + W],
                            zsb[0:64, 0:W].bitcast(f32),
                            rcp[0:64, 0:W], op=Alu.mult)
                        nc.gpsimd.tensor_tensor(
                            zto[0:64, 0:W],
                            zsb[0:64, 512:512 + W].bitcast(f32),
                            rcp[0:64, 512:512 + W], op=Alu.mult)
                        nc.sync.dma_start(
                            yT[64:P, hp, s0:s0 + W], zto[0:64, 0:W])

                    # ---- Phase 2: Q,K projections + per-head-pair
                    # attention. The V projection is spread through the
                    # loop as PE filler so the tensor engine never idles
                    # long enough for HAM to re-throttle while ScalarE
                    # streams the exps. ----
                    def proj(w_sb, dst, bias, m):
                        for (s0, W) in ST:
                            ps = qkpsum.tile([P, 512], f32, tag="ps")
                            for k in range(KT):
                                nc.tensor.matmul(
                                    ps[0:P, 0:W],
                                    w_sb[:, k, m * P:(m + 1) * P],
                                    xT[:, k, s0:s0 + W],
                                    start=(k == 0), stop=(k == KT - 1),
                                )
                            nc.vector.tensor_scalar_add(
                                dst[:, m, s0:s0 + W], ps[0:P, 0:W],
                                bias[:, m:m + 1])

                    def v_unit(nt, tb):
                        n0, Wn = FT[nt]
                        rows = LAST if tb == NB - 1 else P
                        t0 = tb * P
                        ps = qkpsum.tile([P, 512], f32, tag="ps",
                                         name=f"v_ps_{nt}_{tb}")
                        for k in range(KT):
                            nc.tensor.matmul(
                                ps[0:rows, 0:Wn],
                                xT[:, k, t0:t0 + rows],
                                wv_sb[:, k, n0:n0 + Wn],
                                start=(k == 0), stop=(k == KT - 1),
                            )
                        src = ps[0:rows, 0:Wn].rearrange(
                            "p (h e) -> p h e", e=P)
                        hp0 = 4 * nt
                        nc.vector.tensor_copy(
                            v_e[0:rows, hp0:hp0 + 4, tb, 0:64],
                            src[:, :, 0:64])
                        nc.vector.tensor_copy(
                            v_o[0:rows, hp0:hp0 + 4, tb, 0:64],
                            src[:, :, 64:128])

                    # all v slabs up front (dense PE work while the wq/wk
                    # DMAs finish streaming)
                    for tb in range(NB):
                        for nt in range(2):
                            v_unit(nt, tb)
                    # Q runs ahead of K to match the wq/wk DMA arrival order
                    for m in range(4):
                        proj(wq_sb, qT, bq_sb, m)
                    for m in range(KT):
                        proj(wk_sb, kT, bk_sb, m)
                        if m < 4:
                            proj(wq_sb, qT, bq_sb, m + 4)
                        attn(ST[0][0], ST[0][1], m)

                    # ---- Phases 3+4: second-s-tile attention interleaved
                    # with the output projection for s < 512 (its psum
                    # tiles reuse the qkpsum pool). GELU is deferred: raw
                    # psum is staged to SBUF (DVE) and all GELUs run
                    # back-to-back at the end, so the ScalarE activation
                    # table never thrashes between Exp and Gelu. ----
                    with (
                        tc.tile_pool(name="tstage", bufs=1) as tstage,
                        tc.tile_pool(name="outp", bufs=4) as outp,
                    ):
                        tts = tstage.tile([P, NB, 2, 512], bf16)

                        def oproj_mm(sb):
                            rows = LAST if sb == NB - 1 else P
                            r0 = sb * P
                            for ft, (f0, Fw) in enumerate(FT):
                                ps = qkpsum.tile([P, 512], f32, tag="ps")
                                for k in range(KT):
                                    nc.tensor.matmul(
                                        ps[0:rows, 0:Fw],
                                        yT[:, k, r0:r0 + rows],
                                        wp_sb[:, k, f0:f0 + Fw],
                                        start=(k == 0), stop=False,
                                    )
                                nc.tensor.matmul(
                                    ps[0:rows, 0:Fw],
                                    ones_b[0:1, 0:rows],
                                    bp_sb[0:1, f0:f0 + Fw],
                                    start=False, stop=True,
                                )
                                nc.vector.tensor_copy(
                                    tts[0:rows, sb, ft, 0:Fw],
                                    ps[0:rows, 0:Fw])

                        def gelu_flush(sb):
                            rows = LAST if sb == NB - 1 else P
                            r0 = sb * P
                            for ft, (f0, Fw) in enumerate(FT):
                                ot = outp.tile([P, 512], f32, tag="ot")
                                act = AF.Identity if _NO_GELU else AF.Gelu
                                nc.scalar.activation(
                                    ot[0:rows, 0:Fw],
                                    tts[0:rows, sb, ft, 0:Fw], act)
                                nc.sync.dma_start(
                                    out_d[r0:r0 + rows, f0:f0 + Fw],
                                    ot[0:rows, 0:Fw])

                        for hp in range(H // 2):
                            attn(ST[1][0], ST[1][1], hp)
                            if hp >= 4:
                                oproj_mm(hp - 4)
                        for sb in range(4, NB):
                            oproj_mm(sb)
                            gelu_flush(sb - 4)
                            gelu_flush(sb)

    nc.compile()
    return nc


def get_nc():
    if "nc" not in _CACHE:
        _CACHE["nc"] = _build_nc()
    return _CACHE["nc"]


def make_in_maps(inputs):
    import ml_dtypes
    bf16 = ml_dtypes.bfloat16
    f8 = ml_dtypes.float8_e4m3

    x = np.asarray(inputs["x"], np.float32)
    wq = np.asarray(inputs["wq"], np.float32)
    wk = np.asarray(inputs["wk"], np.float32)
    wv = np.asarray(inputs["wv"], np.float32)
    wp = np.asarray(inputs["wp"], np.float32)
    bq = np.asarray(inputs["bq"], np.float32)
    bk = np.asarray(inputs["bk"], np.float32)
    bv = np.asarray(inputs["bv"], np.float32)
    bp = np.asarray(inputs["bp"], np.float32)

    # [H, E, D] -> [E, H*D] (concat head outputs along columns)
    wq2 = np.ascontiguousarray(
        wq.transpose(1, 0, 2).reshape(E, E).astype(bf16))
    wk2 = np.ascontiguousarray(
        wk.transpose(1, 0, 2).reshape(E, E).astype(bf16))
    wv2 = np.ascontiguousarray(
        wv.transpose(1, 0, 2).reshape(E, E).astype(bf16))
    wp2 = np.ascontiguousarray(wp.astype(bf16))
    # per-partition bias layout: bqt[p, m] = bq_flat[m*128 + p]
    bqt = np.ascontiguousarray(bq.reshape(-1).reshape(KT, P).T)
    bkt = np.ascontiguousarray(bk.reshape(-1).reshape(KT, P).T)
    # fold bv into output bias: y = z + bv  =>  out += bv @ wp
    bpe = (bp.astype(np.float64)
           + bv.reshape(-1).astype(np.float64) @ wp.astype(np.float64))
    bpe = np.ascontiguousarray(
        bpe.astype(np.float32).astype(bf16).reshape(1, E))

    shared = {"wq2": wq2, "wk2": wk2, "wv2": wv2, "wp2": wp2,
              "bqt": bqt, "bkt": bkt, "bpe": bpe}
    return [dict(shared, x=np.ascontiguousarray(x[b].astype(bf16)))
            for b in range(B)]


def run(inputs, trace=False):
    from concourse.bass_utils import run_bass_kernel_spmd
    nc = get_nc()
    in_maps = make_in_maps(inputs)
    res = run_bass_kernel_spmd(nc, in_maps, list(range(NCORES)), trace=trace)
    out = np.stack([np.asarray(res.results[i]["out"]) for i in range(NCORES)])
    return out.astype(np.float32), res


def kernel(**inputs):
    out, _ = run(inputs, trace=False)
    return out


# revision 42
# speedup vs baseline: 1.3494x; 1.3494x over previous
"""
Trainium2 Bass kernel for nn_MultiHeadAttention_74586402062628.

Data-parallel across 8 NeuronCores: one batch element per core.

Per-core formulation (B=8, S=1000, E=1024, H=16, D=64), bf16 matmuls:
  - x [S,E] bf16 is transposed on-chip (PE transpose) to xT [E,S] bf16;
    the V projection runs per t-block right behind each block's
    transposes so the PE is dense from the start (HAM warm-up).
  - Q,K projections produce qT,kT [H*D, S] bf16 (head-dim on partitions;
    head h lives in 128-row tile h//2 at partition base (h%2)*64).
    Attention for the first s-tile of head pair m is emitted right after
    projection m so its exp() hides under the projection stretch.
  - V is scattered per-head into [v(64) | ones] slabs so the AV matmul's
    psum row 64 yields the softmax denominator for free.
  - Attention is computed transposed: scoresT[t,s] = k_t . q_s / sqrt(S).
    bf16 matmuls run at 1 cyc/row at ANY free dim, so every scores/AV
    tile is trimmed to exactly the unmasked columns (off = t0-s0). The
    causal diagonal is applied with a gpsimd affine_select (keep c >= p)
    directly on the exp tile; exp() without max-subtraction (logits are
    tiny for this problem).
  - Normalize: zp psum is evicted early to SBUF (frees the accumulator;
    fp32r so the denom row can feed the fp32r ones-matmul broadcast),
    1/denom via reciprocal_approx_fast on the broadcast, multiplies on
    GpSimd; odd heads partition-shift into yT via SBUF->SBUF DMA.
  - Output projection for s<512 is interleaved with second-s-tile
    attention (which is exp-bound on ScalarE); bias row added via a K=1
    ones matmul; exact GELU on ScalarE.
  - bv is folded into an effective output bias bpe = bp + bv @ wp (valid
    because softmax rows sum to 1); bq/bk are added on PSUM->SBUF
    eviction (fp32).
"""

import math
import os
import sys

for _p in ("/opt/trn_rl_repo", "/opt/pypackages"):
    if _p not in sys.path:
        sys.path.insert(0, _p)

import numpy as np

B, S, E, H, D = 8, 1000, 1024, 16, 64
P = 128
NB = 8                      # 128-row blocks covering S (last is partial)
LAST = S - (NB - 1) * P     # 104
KT = 8                      # 128-row contraction tiles covering E
ST = ((0, 512), (512, 488))     # s tiles (start, width) covering S
FT = ((0, 512), (512, 512))     # f/n tiles covering E
SCALE = 1.0 / math.sqrt(S)
NCORES = 8
PIPE = 2

# BASSMHA_NO_GELU=1: replace final GELU with Identity (CoreSim lacks Gelu)
_NO_GELU = os.environ.get("BASSMHA_NO_GELU", "0") == "1"

_CACHE = {}


def _build_nc():
    from concourse import bass, bacc
    import concourse.mybir as mybir
    from concourse import tile
    from concourse.masks import make_identity

    dt = mybir.dt
    f32 = dt.float32
    f32r = dt.float32r
    bf16 = dt.bfloat16
    AF = mybir.ActivationFunctionType
    Alu = mybir.AluOpType

    nc = bacc.Bacc("TRN2", debug=False, target_bir_lowering=False,
                   num_devices=NCORES)

    x_d = nc.declare_dram_parameter("x", [S, E], bf16, isOutput=False)
    wq_d = nc.declare_dram_parameter("wq2", [E, E], bf16, isOutput=False)
    wk_d = nc.declare_dram_parameter("wk2", [E, E], bf16, isOutput=False)
    wv_d = nc.declare_dram_parameter("wv2", [E, E], bf16, isOutput=False)
    wp_d = nc.declare_dram_parameter("wp2", [E, E], bf16, isOutput=False)
    bq_d = nc.declare_dram_parameter("bqt", [P, KT], f32, isOutput=False)
    bk_d = nc.declare_dram_parameter("bkt", [P, KT], f32, isOutput=False)
    bp_d = nc.declare_dram_parameter("bpe", [1, E], bf16, isOutput=False)
    out_d = nc.declare_dram_parameter("out", [S, E], f32, isOutput=True)

    def g2(ap):
        return ap.rearrange("p (g c) -> p g c", g=2)

    with tile.TileContext(nc) as tc:
        with (
            tc.tile_pool(name="const", bufs=1) as constp,
            tc.tile_pool(name="persist", bufs=1) as persist,
        ):
            # identity first on the gpsimd queue — it gates every transpose
            # and DMA triggers occupy the queue for the whole transfer
            ident = constp.tile([P, P], bf16)
            make_identity(nc, ident[:])
            # ones on every partition, f32r for the K=1 denom broadcast
            ones_r = constp.tile([P, P], f32r)
            nc.gpsimd.memset(ones_r[:].bitcast(f32), 1.0)
            # single ones row (partition 0) for the K=1 output-bias matmul
            ones_b = constp.tile([1, P], bf16)
            nc.gpsimd.memset(ones_b[:], 1.0)

            # Persistent activations
            qT = persist.tile([P, KT, S], bf16)      # [hd, m, s]
            kT = persist.tile([P, KT, S], bf16)
            v_e = persist.tile([P, H // 2, NB, 65], bf16)
            v_o = persist.tile([P, H // 2, NB, 65], bf16)
            yT = persist.tile([P, KT, S], bf16)      # normalized z, stacked

            # weights resident in SBUF for the whole kernel, spread across
            # the three DMA-capable queues so wv/wq/wk stream concurrently
            # (wv gates the V projection, wq the Q loop, wk trails)
            wq_sb = persist.tile([P, KT, E], bf16)
            wk_sb = persist.tile([P, KT, E], bf16)
            wv_sb = persist.tile([P, KT, E], bf16)
            wp_sb = persist.tile([P, KT, E], bf16)
            for k in range(KT):
                nc.sync.dma_start(wv_sb[:, k, :], wv_d[k * P:(k + 1) * P, :])
            for k in range(KT):
                nc.scalar.dma_start(wq_sb[:, k, :], wq_d[k * P:(k + 1) * P, :])
            for k in range(KT):
                nc.scalar.dma_start(wk_sb[:, k, :], wk_d[k * P:(k + 1) * P, :])
            for k in range(KT):
                nc.sync.dma_start(wp_sb[:, k, :], wp_d[k * P:(k + 1) * P, :])
            bq_sb = constp.tile([P, KT], f32)
            nc.sync.dma_start(bq_sb[:], bq_d[:, :])
            bk_sb = constp.tile([P, KT], f32)
            nc.sync.dma_start(bk_sb[:], bk_d[:, :])
            bp_sb = constp.tile([1, E], bf16)
            nc.sync.dma_start(bp_sb[:], bp_d[:, :])

            with (
                tc.tile_pool(name="xT", bufs=1) as xtp,
                tc.tile_pool(name="qkpsum", bufs=2, space="PSUM") as qkpsum,
            ):
                xT = xtp.tile([P, KT, S], bf16)

                # ---- Phase 1: transpose x -> xT ----
                with (
                    tc.tile_pool(name="xload", bufs=NB) as xload,
                    tc.tile_pool(name="tpsum", bufs=3, space="PSUM") as tpsum,
                ):
                    # x loads first on the gpsimd queue so they stream in
                    # parallel with the weight DMAs on the sync queue
                    xts = []
                    for sb in range(NB):
                        rows = LAST if sb == NB - 1 else P
                        xt = xload.tile([P, E], bf16, tag="xt",
                                        name=f"xt_{sb}")
                        nc.gpsimd.dma_start(xt[0:rows, :],
                                            x_d[sb * P:sb * P + rows, :])
                        xts.append(xt)
                    # init v slabs: ones columns everywhere, then zero the
                    # tb7 padding rows (96:128; the scatter rewrites rows
                    # 0:104 afterwards)
                    nc.gpsimd.memset(v_e[:, :, :, 64:65], 1.0)
                    nc.gpsimd.memset(v_o[:, :, :, 64:65], 1.0)
                    nc.gpsimd.memset(v_e[96:P, :, NB - 1, :], 0.0)
                    nc.gpsimd.memset(v_o[96:P, :, NB - 1, :], 0.0)
                    for sb in range(NB):
                        rows = LAST if sb == NB - 1 else P
                        t0 = sb * P
                        xt = xts[sb]
                        # all 8 transposes of this block share one psum bank;
                        # a single 2x-rate DVE copy evicts them together
                        tp = tpsum.tile([P, KT, P], bf16, tag="tp")
                        for eb in range(KT):
                            nc.tensor.transpose(
                                tp[0:P, eb, 0:rows],
                                xt[0:rows, eb * P:(eb + 1) * P],
                                ident[0:rows, 0:rows],
                            )
                        nc.vector.tensor_copy(
                            xT[:, :, t0:t0 + rows], tp[:, :, 0:rows])
                # ---- attention pools (live through phases 2-4) ----
                with (
                    tc.tile_pool(name="expp", bufs=5) as expp,
                    tc.tile_pool(name="zsbp", bufs=2) as zsbp,
                    tc.tile_pool(name="rcpp", bufs=2) as rcpp,
                    tc.tile_pool(name="ztop", bufs=2) as ztop,
                    tc.tile_pool(name="spsum", bufs=2, space="PSUM") as spsum,
                    tc.tile_pool(name="zpsum", bufs=1, space="PSUM") as zpsum,
                ):
                    def attn(s0, W, hp):
                        n_tb = (s0 + W + P - 1) // P
                        zp = zpsum.tile([P, 1024], f32, tag="zp")
                        exs = {}
                        geom = {}
                        for tb in range(n_tb):
                            rows = LAST if tb == NB - 1 else P
                            t0 = tb * P
                            off = t0 - s0 if t0 >= s0 else 0
                            geom[tb] = (rows, t0 >= s0, off, W - off)
                        for i in range(n_tb + PIPE):
                            if i < n_tb:
                                tb = i
                                rows, has_diag, off, N = geom[tb]
                                t0 = tb * P
                                sp = spsum.tile([P, 1024], f32, tag="sp")
                                for par in range(2):
                                    base = par * 64
                                    nc.tensor.matmul(
                                        sp[0:rows, 512 * par:512 * par + N],
                                        kT[base:base + 64, hp, t0:t0 + rows],
                                        qT[base:base + 64, hp,
                                           s0 + off:s0 + W],
                                        start=True, stop=True,
                                    )
                                ex = expp.tile([P, 1024], bf16, tag="ex")
                                exv, spv = g2(ex[:, :]), g2(sp[:, :])
                                if rows < P:
                                    nc.vector.memset(exv[96:P, :, 0:N], 0.0)
                                nc.scalar.activation(
                                    exv[0:rows, :, 0:N], spv[0:rows, :, 0:N],
                                    AF.Exp, scale=SCALE)
                                if has_diag:
                                    dw = min(rows, N)
                                    nc.gpsimd.affine_select(
                                        out=exv[0:rows, :, 0:dw],
                                        in_=exv[0:rows, :, 0:dw],
                                        compare_op=Alu.is_ge, fill=0.0,
                                        base=0, channel_multiplier=-1,
                                        pattern=[[0, 2], [1, dw]],
                                    )
                                exs[tb] = ex
                            j = i - PIPE
                            if 0 <= j < n_tb:
                                rows, has_diag, off, N = geom[j]
                                ex = exs.pop(j)
                                for par, vs in ((0, v_e), (1, v_o)):
                                    nc.tensor.matmul(
                                        zp[0:65,
                                           512 * par + off:512 * par + W],
                                        vs[:, hp, j, 0:65],
                                        ex[0:P, 512 * par:512 * par + N],
                                        start=(j == 0), stop=(j == n_tb - 1),
                                        skip_group_check=True,
                                    )
                        # normalize: evict zp to SBUF (frees the accumulator;
                        # f32r so the denom row feeds the f32r ones-matmul),
                        # broadcast denom to partitions 0:64, fast reciprocal
                        # on the broadcast, multiply on GpSimd.
                        zsb = zsbp.tile([65, 1024], f32r, tag="zsb")
                        with nc.allow_low_precision(
                                reason="z/denom round to fp32r for the "
                                "broadcast matmul; within tolerance"):
                            nc.vector.tensor_copy(
                                g2(zsb[:, :])[0:65, :, 0:W],
                                g2(zp[:, :])[0:65, :, 0:W])
                        # bc reuses zp's banks (ring WAR dep on the copy)
                        bc = zpsum.tile([P, 1024], f32, tag="zp")
                        for par in range(2):
                            nc.tensor.matmul(
                                bc[0:64, 512 * par:512 * par + W],
                                ones_r[64:65, 0:64],
                                zsb[64:65, 512 * par:512 * par + W],
                                start=True, stop=True)
                        rcp = rcpp.tile([64, 1024], f32, tag="rcp")
                        for par in range(2):
                            nc.vector.reciprocal_approx_fast(
                                rcp[0:64, 512 * par:512 * par + W],
                                bc[0:64, 512 * par:512 * par + W])
                        nc.gpsimd.tensor_tensor(
                            yT[0:64, hp, s0:s0 + W],
                            zsb[0:64, 0:W].bitcast(f32),
                            rcp[0:64, 0:W], op=Alu.mult)
                        zto = ztop.tile([64, 512], bf16, tag="zto")
                        nc.gpsimd.tensor_tensor(
                            zto[0:64, 0:W],
                            zsb[0:64, 512:512 + W].bitcast(f32),
                            rcp[0:64, 512:512 + W], op=Alu.mult)
                        nc.sync.dma_start(
                            yT[64:P, hp, s0:s0 + W], zto[0:64, 0:W])

                    # ---- Phase 2: Q,K projections + per-head-pair
                    # attention. The V projection is spread through the
                    # loop as PE filler so the tensor engine never idles
                    # long enough for HAM to re-throttle while ScalarE
                    # streams the exps. ----
                    def proj(w_sb, dst, bias, m):
                        for (s0, W) in ST:
                            ps = qkpsum.tile([P, 512], f32, tag="ps")
                            for k in range(KT):
                                nc.tensor.matmul(
                                    ps[0:P, 0:W],
                                    w_sb[:, k, m * P:(m + 1) * P],
                                    xT[:, k, s0:s0 + W],
                                    start=(k == 0), stop=(k == KT - 1),
                                )
                            nc.vector.tensor_scalar_add(
                                dst[:, m, s0:s0 + W], ps[0:P, 0:W],
                                bias[:, m:m + 1])

                    def v_unit(nt, tb):
                        n0, Wn = FT[nt]
                        rows = LAST if tb == NB - 1 else P
                        t0 = tb * P
                        ps = qkpsum.tile([P, 512], f32, tag="ps",
                                         name=f"v_ps_{nt}_{tb}")
                        for k in range(KT):
                            nc.tensor.matmul(
                                ps[0:rows, 0:Wn],
                                xT[:, k, t0:t0 + rows],
                                wv_sb[:, k, n0:n0 + Wn],
                                start=(k == 0), stop=(k == KT - 1),
                            )
                        src = ps[0:rows, 0:Wn].rearrange(
                            "p (h e) -> p h e", e=P)
                        hp0 = 4 * nt
                        nc.vector.tensor_copy(
                            v_e[0:rows, hp0:hp0 + 4, tb, 0:64],
                            src[:, :, 0:64])
                        nc.vector.tensor_copy(
                            v_o[0:rows, hp0:hp0 + 4, tb, 0:64],
                            src[:, :, 64:128])

                    # all v slabs up front (dense PE work while the wq/wk
                    # DMAs finish streaming)
                    for tb in range(NB):
                        for nt in range(2):
                            v_unit(nt, tb)
                    # Q runs ahead of K to match the wq/wk DMA arrival order
                    for m in range(4):
                        proj(wq_sb, qT, bq_sb, m)
                    for m in range(KT):
                        proj(wk_sb, kT, bk_sb, m)
                        if m < 4:
                            proj(wq_sb, qT, bq_sb, m + 4)
                        attn(ST[0][0], ST[0][1], m)

                    # ---- Phases 3+4: second-s-tile attention interleaved
                    # with the output projection for s < 512 (its psum
                    # tiles reuse the qkpsum pool). GELU is deferred: raw
                    # psum is staged to SBUF (DVE) and all GELUs run
                    # back-to-back at the end, so the ScalarE activation
                    # table never thrashes between Exp and Gelu. ----
                    with (
                        tc.tile_pool(name="tstage", bufs=1) as tstage,
                        tc.tile_pool(name="outp", bufs=4) as outp,
                    ):
                        tts = tstage.tile([P, NB, 2, 512], bf16)

                        def oproj_mm(sb):
                            rows = LAST if sb == NB - 1 else P
                            r0 = sb * P
                            for ft, (f0, Fw) in enumerate(FT):
                                ps = qkpsum.tile([P, 512], f32, tag="ps")
                                for k in range(KT):
                                    nc.tensor.matmul(
                                        ps[0:rows, 0:Fw],
                                        yT[:, k, r0:r0 + rows],
                                        wp_sb[:, k, f0:f0 + Fw],
                                        start=(k == 0), stop=False,
                                    )
                                nc.tensor.matmul(
                                    ps[0:rows, 0:Fw],
                                    ones_b[0:1, 0:rows],
                                    bp_sb[0:1, f0:f0 + Fw],
                                    start=False, stop=True,
                                )
                                nc.vector.tensor_copy(
                                    tts[0:rows, sb, ft, 0:Fw],
                                    ps[0:rows, 0:Fw])

                        def gelu_flush(sb):
                            rows = LAST if sb == NB - 1 else P
                            r0 = sb * P
                            for ft, (f0, Fw) in enumerate(FT):
                                ot = outp.tile([P, 512], f32, tag="ot")
                                act = AF.Identity if _NO_GELU else AF.Gelu
                                nc.scalar.activation(
                                    ot[0:rows, 0:Fw],
                                    tts[0:rows, sb, ft, 0:Fw], act)
                                nc.sync.dma_start(
                                    out_d[r0:r0 + rows, f0:f0 + Fw],
                                    ot[0:rows, 0:Fw])

                        for hp in range(H // 2):
                            attn(ST[1][0], ST[1][1], hp)
                            if hp >= 4:
                                oproj_mm(hp - 4)
                        for sb in range(4, NB):
                            oproj_mm(sb)
                            gelu_flush(sb - 4)
                            gelu_flush(sb)

    nc.compile()
    return nc


def get_nc():
    if "nc" not in _CACHE:
        _CACHE["nc"] = _build_nc()
    return _CACHE["nc"]


def make_in_maps(inputs):
    import ml_dtypes
    bf16 = ml_dtypes.bfloat16
    f8 = ml_dtypes.float8_e4m3

    x = np.asarray(inputs["x"], np.float32)
    wq = np.asarray(inputs["wq"], np.float32)
    wk = np.asarray(inputs["wk"], np.float32)
    wv = np.asarray(inputs["wv"], np.float32)
    wp = np.asarray(inputs["wp"], np.float32)
    bq = np.asarray(inputs["bq"], np.float32)
    bk = np.asarray(inputs["bk"], np.float32)
    bv = np.asarray(inputs["bv"], np.float32)
    bp = np.asarray(inputs["bp"], np.float32)

    # [H, E, D] -> [E, H*D] (concat head outputs along columns)
    wq2 = np.ascontiguousarray(
        wq.transpose(1, 0, 2).reshape(E, E).astype(bf16))
    wk2 = np.ascontiguousarray(
        wk.transpose(1, 0, 2).reshape(E, E).astype(bf16))
    wv2 = np.ascontiguousarray(
        wv.transpose(1, 0, 2).reshape(E, E).astype(bf16))
    wp2 = np.ascontiguousarray(wp.astype(bf16))
    # per-partition bias layout: bqt[p, m] = bq_flat[m*128 + p]
    bqt = np.ascontiguousarray(bq.reshape(-1).reshape(KT, P).T)
    bkt = np.ascontiguousarray(bk.reshape(-1).reshape(KT, P).T)
    # fold bv into output bias: y = z + bv  =>  out += bv @ wp
    bpe = (bp.astype(np.float64)
           + bv.reshape(-1).astype(np.float64) @ wp.astype(np.float64))
    bpe = np.ascontiguousarray(
        bpe.astype(np.float32).astype(bf16).reshape(1, E))

    shared = {"wq2": wq2, "wk2": wk2, "wv2": wv2, "wp2": wp2,
              "bqt": bqt, "bkt": bkt, "bpe": bpe}
    return [dict(shared, x=np.ascontiguousarray(x[b].astype(bf16)))
            for b in range(B)]


def run(inputs, trace=False):
    from concourse.bass_utils import run_bass_kernel_spmd
    nc = get_nc()
    in_maps = make_in_maps(inputs)
    res = run_bass_kernel_spmd(nc, in_maps, list(range(NCORES)), trace=trace)
    out = np.stack([np.asarray(res.results[i]["out"]) for i in range(NCORES)])
    return out.astype(np.float32), res


def kernel(**inputs):
    out, _ = run(inputs, trace=False)
    return out
